# revision 1
# baseline (speedup 1.0000x reference)
"""LSTM encoder kernel for Trainium2 (Bass/Tile), data-parallel over batch on 8 cores.

Math (per core, batch shard B=256):
  z_t = Wcat @ [x_t ; hh_{t-1}] + b      (gates pre-activation, [128, B])
  Wcat = [Wx ; 2*Whh] with g-gate columns additionally scaled by 2 so a single
  sigmoid over all 128 gate rows yields S_g = sigmoid(2 z_g), i.e.
  tanh(z_g) = 2 S_g - 1.  Reparametrize cc = c/2, hh = h/2:
    t1 = S_g - 1/2
    u  = t1 * S_i          = (i*g)/2
    v  = S_f * cc          = (f*c)/2
    cc = v + u             = c_new/2
    S_c = sigmoid(4*cc)    = sigmoid(2*c_new)
    hh = (S_c - 1/2) * S_o = o*tanh(c_new)/2 = h/2
  Host multiplies the stored hh history by 2 to recover h.

Layouts: gates on partitions (128), batch on free dim. Per chunk of TC steps one
SBUF tile [42, TC*B] holds rhs slots [x_t ; hh_{t-1}]; the hh write of step t
lands in slot t+1 (next chunk's slot 0 at boundaries). Output DMA reads rows
10:42. Partition-start rule (both-SBUF operands must share start): S_g is
relocated to start 0 (t1, GPSIMD), cc lives at start 32 (pairs with f), sigma_c
output is placed at start 96 (pairs with o). DVE runs u/cc/hh, GPSIMD runs t1/v.
"""

import numpy as np
from contextlib import ExitStack

import concourse.bass as bass
import concourse.tile as tile
from concourse import bacc, mybir
from concourse.bass_utils import run_bass_kernel_spmd

T_FULL = 512
B_FULL = 2048
IN = 10
H = 32
G = 4 * H          # 128 gate rows
K = IN + H         # 42 contraction rows of the combined matmul
NCORES = 8
B = B_FULL // NCORES  # 256 batch per core

NB = 2          # batch sub-blocks per core (latency pipelining)
FD = B // NB    # free-dim per block
TC = 16         # timesteps per SBUF chunk

DT = mybir.dt.float32
SIG = mybir.ActivationFunctionType.Sigmoid
MULT = mybir.AluOpType.mult
ADD = mybir.AluOpType.add
SUB = mybir.AluOpType.subtract

_CACHE = {}


def _build(t_total=T_FULL, tc=TC, nb=NB):
    fd = B // nb
    nchunk = t_total // tc
    nc = bacc.Bacc(trn_type="TRN2", debug=False, target_bir_lowering=False)

    xT = nc.dram_tensor("xT", [t_total, IN, B], DT, kind="ExternalInput").ap()
    wcat = nc.dram_tensor("wcat", [K, G], DT, kind="ExternalInput").ap()
    bg = nc.dram_tensor("bg", [G, 1], DT, kind="ExternalInput").ap()
    hout = nc.dram_tensor("hout", [t_total, H, B], DT, kind="ExternalOutput").ap()

    with tile.TileContext(nc) as tc_, ExitStack() as ctx:
        const = ctx.enter_context(tc_.tile_pool(name="const", bufs=1))
        xpool = ctx.enter_context(tc_.tile_pool(name="xpool", bufs=3))
        spool = ctx.enter_context(tc_.tile_pool(name="spool", bufs=4))
        cpool = ctx.enter_context(tc_.tile_pool(name="cpool", bufs=3))
        tpool = ctx.enter_context(tc_.tile_pool(name="tpool", bufs=6))
        pspool = ctx.enter_context(tc_.tile_pool(name="pspool", bufs=4, space="PSUM"))

        w_t = const.tile([K, G], DT)
        nc.sync.dma_start(w_t[:], wcat)
        bg_t = const.tile([G, 1], DT)
        nc.sync.dma_start(bg_t[:], bg)

        # rhs chunk tiles: [K, tc*B]; rows 0:H = hh slots, rows H:K = x slots
        # (hh first so its partition start is 0; x lands at start 32)
        chunk_tiles = {}

        def get_chunk(ch):
            if ch not in chunk_tiles:
                t = xpool.tile([K, tc * B], DT, name="rhs", tag="rhs")
                if ch < nchunk:
                    nc.sync.dma_start(
                        t[H:K].rearrange("p (t b) -> p t b", t=tc),
                        xT[ch * tc:(ch + 1) * tc].rearrange("t p b -> p t b"),
                    )
                chunk_tiles[ch] = t
            return chunk_tiles[ch]

        cur = get_chunk(0)
        # hh_{-1} = 0
        nc.vector.memset(cur[0:H, 0:B], 0.0)

        c_prev = []
        for blk in range(nb):
            c0 = cpool.tile([2 * H, fd], DT, name=f"cc{blk}", tag=f"cc{blk}")
            nc.vector.memset(c0[H:2 * H], 0.0)
            c_prev.append(c0)

        # Rotated software pipeline: block 1 runs half a step behind block 0,
        # so each block's PE/ACT roundtrip hides inside the other's DVE work.
        # Phase A(b, s): mm -> sigma_all -> v(GPSIMD) -> t1 -> u
        # Phase B(b, s): cc -> sigma_c -> hh
        # Tick s: A(b1, s), B(b0, s), A(b0, s+1), B(b1, s)
        state = {}

        def phase_a(blk, s_global):
            ch_, s_ = divmod(s_global, tc)
            col = s_ * B + blk * fd
            rhs = get_chunk(ch_)
            p = pspool.tile([G, fd], mybir.dt.float32, name="gates",
                            tag=f"gates{blk}")
            nc.tensor.matmul(p[:], w_t[:], rhs[:, col:col + fd],
                             start=True, stop=True)
            s_t = spool.tile([G, fd], DT, name="sgm", tag=f"sgm{blk}")
            nc.scalar.activation(s_t[:], p[:], SIG, bias=bg_t[:])
            # v = f * cc at start 32 (GPSIMD, off the DVE chain)
            v = tpool.tile([2 * H, fd], DT, name="v", tag=f"v{blk}")
            nc.gpsimd.tensor_tensor(
                v[H:2 * H], s_t[H:2 * H], c_prev[blk][H:2 * H], MULT)
            # t1 = S_g - 0.5 relocated to start 0 (DVE ts)
            t1 = tpool.tile([H, fd], DT, name="t1", tag=f"t1{blk}")
            nc.vector.tensor_scalar(t1[:], s_t[2 * H:3 * H], 0.5, None, SUB)
            # u = t1 * i (both at start 0), placed at start 32
            u = tpool.tile([2 * H, fd], DT, name="u", tag=f"u{blk}")
            nc.vector.tensor_tensor(u[H:2 * H], t1[:], s_t[0:H], MULT)
            state[blk] = (s_t, u, v, s_global)

        def phase_b(blk):
            s_t, u, v, s_global = state[blk]
            c_new = cpool.tile([2 * H, fd], DT, name=f"ccn{blk}",
                               tag=f"cc{blk}")
            nc.vector.tensor_tensor(c_new[H:2 * H], v[H:2 * H],
                                    u[H:2 * H], ADD)
            c_prev[blk] = c_new
            # sc = sigmoid(4*cc) relocated to start 96 (pairs with o)
            sc = spool.tile([G, fd], DT, name="sc", tag=f"sc{blk}")
            nc.scalar.activation(sc[3 * H:4 * H], c_new[H:2 * H],
                                 SIG, scale=4.0)
            ch_, s_ = divmod(s_global + 1, tc)
            col = s_ * B + blk * fd
            hdst = get_chunk(ch_)[0:H, col:col + fd]
            nc.vector.scalar_tensor_tensor(
                hdst, sc[3 * H:4 * H], 0.5, s_t[3 * H:4 * H], SUB, MULT)

        def emit_out(ch):
            cur_, nxt_ = get_chunk(ch), get_chunk(ch + 1)
            nc.sync.dma_start(
                hout[ch * tc:ch * tc + tc - 1].rearrange("t p b -> p t b"),
                cur_[0:H, B:].rearrange("p (t b) -> p t b", t=tc - 1),
            )
            nc.sync.dma_start(hout[ch * tc + tc - 1], nxt_[0:H, 0:B])

        phase_a(0, 0)
        for s in range(t_total):
            phase_a(1, s)
            phase_b(0)
            if s + 1 < t_total:
                phase_a(0, s + 1)
            phase_b(1)
            if s % tc == tc - 1:
                emit_out(s // tc)
    nc.compile()
    return nc


def _prep_weights(W_emb, b_emb, W_ih, W_hh, b_ih, b_hh):
    f8 = lambda a: np.asarray(a, np.float64)
    Wx = f8(W_ih) @ f8(W_emb)                                  # [G, IN]
    bgv = f8(W_ih) @ f8(b_emb) + f8(b_ih) + f8(b_hh)           # [G]
    wc = np.concatenate([2.0 * f8(W_hh).T, Wx.T], axis=0)      # [K, G] = [hh; x]
    wc[:, 2 * H:3 * H] *= 2.0
    bgv = bgv.copy()
    bgv[2 * H:3 * H] *= 2.0
    return (np.ascontiguousarray(wc.astype(np.float32)),
            np.ascontiguousarray(bgv.astype(np.float32).reshape(G, 1)))


def _run(x, W_emb, b_emb, W_ih, W_hh, b_ih, b_hh, trace=False):
    t_total = x.shape[0]
    key = (t_total, TC, NB)
    if key not in _CACHE:
        _CACHE[key] = _build(t_total, TC, NB)
    nc = _CACHE[key]

    wc, bgv = _prep_weights(W_emb, b_emb, W_ih, W_hh, b_ih, b_hh)
    x = np.asarray(x, np.float32)
    in_maps = []
    for c in range(NCORES):
        xs = np.ascontiguousarray(
            x[:, c * B:(c + 1) * B, :].transpose(0, 2, 1))  # [T, IN, B]
        in_maps.append({"xT": xs, "wcat": wc, "bg": bgv})

    res = run_bass_kernel_spmd(nc, in_maps, list(range(NCORES)), trace=trace)
    out = np.empty((t_total, B_FULL, H), np.float32)
    for c in range(NCORES):
        out[:, c * B:(c + 1) * B, :] = (
            res.results[c]["hout"].transpose(0, 2, 1) * np.float32(2.0))
    return out, res


def kernel(x, W_emb, b_emb, W_ih, W_hh, b_ih, b_hh):
    out, _ = _run(x, W_emb, b_emb, W_ih, W_hh, b_ih, b_hh, trace=False)
    return out

